# revision 16
# baseline (speedup 1.0000x reference)
"""GATv2 3-layer GNN (nn_GCN_10917806866525) on 8 TRN2 NeuronCores.

Sharding: nodes split 12500/core (edge-cut by dst). Per layer, per core:
  A. node-transform GEMM for the local shard (merged [Wl|Wr], act-stationary
     bf16 matmul; stationary tiles packed into partition halves/quarters for
     layers 2/3) -> xl shard + local xr table
  B. AllGather of the xl shard -> full xl table [100352, 64] f32
  C. edge phase over 4096-token windows (host-packed so every window has
     unique dst -> dma_scatter_add duplicate-safety), processed in 2-window
     groups: 2 dma_gathers/window (xl[src], xr[dst]), LeakyReLU via
     scale-copy+max (ACT Lrelu mis-evaluates on HW), att-dot, exp written
     straight into the payload's w column, scatter of [w*xl | w] into 2
     rotating DRAM accumulators
  D. merge accs in 7-tile batches, divide by the w column (softmax
     denominator; max-subtraction skipped, exponents small in fp32),
     + (bl+bias), BN stats via ones-matmul + AllReduce [1,2H], spill pre-BN
     out to acc0
  E. reload, normalize+ReLU (batched), paired PE transposes -> next layer's
     stationary operand
Pooling: per-tile indicator matmuls accumulated in PSUM -> [64,17] partials
per core; host sums cores, divides counts, applies the final linear layer.

Everything is sized to minimize instruction count: the axon execute path
costs ~35-65us per instruction, dwarfing actual engine time."""

import time

import numpy as np
import ml_dtypes

import concourse.bacc as bacc
import concourse.bass as bass
import concourse.mybir as mybir
from concourse import tile
from concourse.bass_utils import run_bass_kernel_spmd

FP32 = mybir.dt.float32
BF16 = mybir.dt.bfloat16
I16 = mybir.dt.int16

NCORES = 8
N = 100000
F = 128
E = 1600000
G = 64
EPS = 1e-5
NEG_SLOPE = 0.2

SHARD = N // NCORES          # 12500
PADSH = 12544                # 98 * 128
NT = PADSH // 128            # 98 node tiles per shard
NB = 14                      # 7-tile epilogue batches
TPB = 7                      # tiles per batch
TABN = NCORES * PADSH        # 100352 rows in the all-gathered xl table
BLOCK = 32768                # int16 gather block
NBLK = (TABN + BLOCK - 1) // BLOCK   # 4
WCAP = 4096                  # tokens per window (= one gather/scatter op)
WT = WCAP // 128             # 32 token columns per window
RACC = 2                     # rotating scatter accumulators
N0 = float(N)

LAYERS = [(128, 64), (64, 32), (32, 16)]  # (F_in, H)
TW = 64                      # gather table width (f32, 256B rows)
ACCW = 128                   # accumulator row stride (512B)

LAST_EXEC_NS = None


# ---------------------------------------------------------------- host prep

def _wrap16(idx):
    """[n] int -> [16, n/16] int16 wrapped (idx j at [j%16, j//16])."""
    return np.ascontiguousarray(np.asarray(idx, np.int16).reshape(-1, 16).T)


def _build_windows(edge_index):
    """Partition edges by dst core; per core, bucket by src table block and
    pack into 4096-token windows with per-window-unique dst_local. Dummy
    tokens (src row 0 of the bucket, dst pad row 12500) fill windows so all
    cores share one SPMD-identical window layout."""
    src = np.concatenate([edge_index[0], np.arange(N, dtype=np.int64)])
    dst = np.concatenate([edge_index[1], np.arange(N, dtype=np.int64)])
    core = dst // SHARD
    dst_local = (dst - core * SHARD).astype(np.int64)
    src_tab = (src // SHARD) * PADSH + (src % SHARD)
    bucket = src_tab // BLOCK
    src_blk = (src_tab - bucket * BLOCK).astype(np.int64)

    per = [[None] * NBLK for _ in range(NCORES)]
    for c in range(NCORES):
        mc = core == c
        for b in range(NBLK):
            m = mc & (bucket == b)
            per[c][b] = (src_blk[m], dst_local[m])

    wb = []
    for b in range(NBLK):
        need = 0
        for c in range(NCORES):
            s, d = per[c][b]
            cnt = np.bincount(d, minlength=SHARD)
            need = max(need, int(np.ceil(len(d) / (WCAP - 96))) + 1, int(cnt.max()))
        wb.append(need)

    srcidx = [[] for _ in range(NCORES)]
    dstidx = [[] for _ in range(NCORES)]
    win_bucket = []
    for b in range(NBLK):
        W = wb[b]
        win_bucket += [b] * W
        for c in range(NCORES):
            s, d = per[c][b]
            order = np.argsort(d, kind="stable")
            s, d = s[order], d[order]
            occ = np.arange(len(d)) - np.searchsorted(d, d, side="left")
            w_of = (d + occ) % W
            counts = np.bincount(w_of, minlength=W)
            if counts.max() > WCAP:
                has = np.zeros((W, SHARD + 1), bool)
                has[w_of, d] = True
                for w in np.where(counts > WCAP)[0]:
                    idxs = np.where(w_of == w)[0]
                    for i in idxs[WCAP:]:
                        cand = np.where((counts < WCAP) & ~has[:, d[i]])[0]
                        if len(cand) == 0:
                            raise RuntimeError("window spill placement failed")
                        has[w, d[i]] = False
                        has[cand[0], d[i]] = True
                        counts[w] -= 1
                        counts[cand[0]] += 1
                        w_of[i] = cand[0]
            o2 = np.lexsort((d, w_of))
            s2, d2, w2 = s[o2], d[o2], w_of[o2]
            starts = np.concatenate([[0], np.cumsum(counts)[:-1]])
            pos = np.arange(len(d2)) - np.repeat(starts, counts)
            out_s = np.zeros((W, WCAP), np.int64)
            out_d = np.full((W, WCAP), SHARD, np.int64)
            out_s[w2, pos] = s2
            out_d[w2, pos] = d2
            srcidx[c].append(out_s.reshape(-1))
            dstidx[c].append(out_d.reshape(-1))

    src_w = [_wrap16(np.concatenate(srcidx[c])) for c in range(NCORES)]
    dst_w = [_wrap16(np.concatenate(dstidx[c])) for c in range(NCORES)]
    return src_w, dst_w, win_bucket


def _make_in_maps(inputs):
    x = np.asarray(inputs["x"], np.float32)
    ei = np.asarray(inputs["edge_index"], np.int64)
    batch = np.asarray(inputs["batch"], np.int64)
    src_w, dst_w, win_bucket = _build_windows(ei)

    xT = np.zeros((NCORES, 128, PADSH), ml_dtypes.bfloat16)
    xt_full = np.ascontiguousarray(x.T).astype(ml_dtypes.bfloat16)
    for c in range(NCORES):
        xT[c, :, :SHARD] = xt_full[:, c * SHARD:(c + 1) * SHARD]

    ident = np.eye(128, dtype=ml_dtypes.bfloat16)
    iota = np.tile(np.arange(G, dtype=np.float32)[None, :], (128, 1))
    batchf = np.full((NCORES, 128, NT), -1.0, np.float32)
    for c in range(NCORES):
        loc = batch[c * SHARD:(c + 1) * SHARD].astype(np.float32)
        pad = np.full(PADSH - SHARD, -1.0, np.float32)
        batchf[c] = np.concatenate([loc, pad]).reshape(NT, 128).T

    const_maps = {"ident": ident, "iota": iota}
    for li, (fi, h) in enumerate(LAYERS):
        Wl = np.asarray(inputs[f"Wl{li+1}"], np.float32)
        bl = np.asarray(inputs[f"bl{li+1}"], np.float32)
        Wr = np.asarray(inputs[f"Wr{li+1}"], np.float32)
        br = np.asarray(inputs[f"br{li+1}"], np.float32)
        att = np.asarray(inputs[f"att{li+1}"], np.float32)
        bias = np.asarray(inputs[f"bias{li+1}"], np.float32)
        gam = np.asarray(inputs[f"gamma{li+1}"], np.float32)
        bet = np.asarray(inputs[f"beta{li+1}"], np.float32)
        wcat = np.concatenate([Wl, Wr], 1).astype(ml_dtypes.bfloat16)  # [fi, 2h]
        rep = 128 // fi
        const_maps[f"wcat{li}"] = np.ascontiguousarray(np.tile(wcat, (rep, 1)))
        const_maps[f"attr{li}"] = np.tile(att[None, :], (128, 1)).astype(np.float32)
        const_maps[f"bout{li}"] = np.tile((bl + bias)[None, :], (128, 1)).astype(np.float32)
        bxr = np.concatenate([np.zeros(h, np.float32), bl + br])
        const_maps[f"bxr{li}"] = np.tile(bxr[None, :], (128, 1)).astype(np.float32)
        const_maps[f"gam{li}"] = gam[None, :].astype(np.float32)
        const_maps[f"bet{li}"] = bet[None, :].astype(np.float32)

    padmask = (np.arange(128) + (NT - 1) * 128 < SHARD).astype(np.float32)[:, None]
    in_maps = [
        {"xT": xT[c], "sidx": src_w[c], "didx": dst_w[c], "batchf": batchf[c],
         "padmask": padmask, **const_maps}
        for c in range(NCORES)
    ]
    return in_maps, win_bucket


# ---------------------------------------------------------------- device nc

def _build_nc(win_bucket):
    TOTW = len(win_bucket)
    TOT16 = TOTW * WCAP // 16
    bstart = {}
    for w, b in enumerate(win_bucket):
        if b not in bstart:
            bstart[b] = [w, 0]
        bstart[b][1] += 1

    nc = bacc.Bacc("TRN2", target_bir_lowering=False, debug=False,
                   num_devices=NCORES)

    xT = nc.declare_dram_parameter("xT", [128, PADSH], BF16, isOutput=False)
    sidx_in = nc.declare_dram_parameter("sidx", [16, TOT16], I16, isOutput=False)
    didx_in = nc.declare_dram_parameter("didx", [16, TOT16], I16, isOutput=False)
    ident_in = nc.declare_dram_parameter("ident", [128, 128], BF16, isOutput=False)
    iota_in = nc.declare_dram_parameter("iota", [128, G], FP32, isOutput=False)
    batch_in = nc.declare_dram_parameter("batchf", [128, NT], FP32, isOutput=False)
    padmask_in = nc.declare_dram_parameter("padmask", [128, 1], FP32, isOutput=False)
    wcat_in, attr_in, bout_in, bxr_in, gam_in, bet_in = [], [], [], [], [], []
    for li, (fi, h) in enumerate(LAYERS):
        wcat_in.append(nc.declare_dram_parameter(f"wcat{li}", [128, 2 * h], BF16, isOutput=False))
        attr_in.append(nc.declare_dram_parameter(f"attr{li}", [128, h], FP32, isOutput=False))
        bout_in.append(nc.declare_dram_parameter(f"bout{li}", [128, h], FP32, isOutput=False))
        bxr_in.append(nc.declare_dram_parameter(f"bxr{li}", [128, 2 * h], FP32, isOutput=False))
        gam_in.append(nc.declare_dram_parameter(f"gam{li}", [1, h], FP32, isOutput=False))
        bet_in.append(nc.declare_dram_parameter(f"bet{li}", [1, h], FP32, isOutput=False))
    pool_out = nc.declare_dram_parameter("pool", [G, 17], FP32, isOutput=True)

    xl_bounce = nc.dram_tensor("xl_bounce", [PADSH, TW], FP32)
    xl_full = nc.dram_tensor("xl_full", [TABN, TW], FP32, addr_space="Shared")
    xr_tab = nc.dram_tensor("xr_tab", [PADSH, TW], FP32)
    accs = [nc.dram_tensor(f"acc{r}", [PADSH, ACCW], FP32) for r in range(RACC)]
    sidx_rep = nc.dram_tensor("sidx_rep", [128, TOT16], I16)
    didx_rep = nc.dram_tensor("didx_rep", [128, TOT16], I16)
    bn_in = nc.dram_tensor("bn_in", [1, 128], FP32)
    bn_out = nc.dram_tensor("bn_out", [1, 128], FP32, addr_space="Shared")

    blk_rows = [min(BLOCK, TABN - b * BLOCK) for b in range(NBLK)]

    def bat(dram, g):
        """rows g*896.. as [128, 7, w] (node (7g+j)*128+p -> [p, j, :])."""
        return dram[g * 896:(g + 1) * 896, :].rearrange("(a p) c -> p a c", p=128)

    with tile.TileContext(nc) as tc:
        with (
            tc.tile_pool(name="persist", bufs=1) as pp,
            tc.tile_pool(name="gemm", bufs=3) as gp,
            tc.tile_pool(name="gpsum", bufs=2, space="PSUM") as gpp,
            tc.tile_pool(name="spsum", bufs=1, space="PSUM") as spp,
            tc.tile_pool(name="win", bufs=2) as wp,
            tc.tile_pool(name="ixp", bufs=1) as ixp,
            tc.tile_pool(name="ep", bufs=2) as ep,
            tc.tile_pool(name="misc", bufs=2) as mp,
        ):
            # ---- persistent loads / setup
            ident_sb = pp.tile([128, 128], BF16)
            nc.sync.dma_start(out=ident_sb[:], in_=ident_in[:])
            iota_sb = pp.tile([128, G], FP32)
            nc.sync.dma_start(out=iota_sb[:], in_=iota_in[:])
            batch_sb = pp.tile([128, NT], FP32)
            nc.sync.dma_start(out=batch_sb[:], in_=batch_in[:])
            padmask_sb = pp.tile([128, 1], FP32)
            nc.sync.dma_start(out=padmask_sb[:], in_=padmask_in[:])
            ones_sb = pp.tile([128, 1], FP32)
            nc.vector.memset(ones_sb[:], 1.0)
            ones_row = pp.tile([1, 128], FP32)
            nc.vector.memset(ones_row[:], 1.0)
            zero_sb = pp.tile([128, 1792], FP32)
            nc.vector.memset(zero_sb[:], 0.0)
            z3 = zero_sb[:].rearrange("p (a b) -> p a b", b=128)
            for k in range(8):
                nc.sync.dma_start(out=sidx_rep[16 * k:16 * (k + 1), :], in_=sidx_in[:])
                nc.sync.dma_start(out=didx_rep[16 * k:16 * (k + 1), :], in_=didx_in[:])

            hT_sb = pp.tile([128, (NT // 2 + 1) * 128], BF16)
            h_bf = pp.tile([128, NT * TW], BF16, tag="hshare")
            hpool_sb = pp.tile([128, NT * 17], FP32, tag="hshare")

            for li, (fi, h) in enumerate(LAYERS):
                h2 = 2 * h
                EL = h + 1
                rep = 128 // fi
                # ---- layer consts
                wcat_sb = mp.tile([128, h2], BF16, tag="wcat")
                nc.sync.dma_start(out=wcat_sb[:], in_=wcat_in[li][:])
                attr_sb = mp.tile([128, h], FP32, tag="attr")
                nc.sync.dma_start(out=attr_sb[:], in_=attr_in[li][:])
                bout_sb = mp.tile([128, h], FP32, tag="bout")
                nc.sync.dma_start(out=bout_sb[:], in_=bout_in[li][:])
                bxr_sb = mp.tile([128, h2], FP32, tag="bxr")
                nc.sync.dma_start(out=bxr_sb[:], in_=bxr_in[li][:])
                gam_sb = mp.tile([1, h], FP32, tag="gam")
                nc.sync.dma_start(out=gam_sb[:], in_=gam_in[li][:])
                bet_sb = mp.tile([1, h], FP32, tag="bet")
                nc.sync.dma_start(out=bet_sb[:], in_=bet_in[li][:])

                # ---- A: GEMM -> xl_bounce + xr_tab (7-tile output batches)
                for g in range(NB):
                    sb = gp.tile([128, TPB * 128], FP32, tag="gemm_sb")
                    sb3 = sb[:].rearrange("p (a b) -> p a b", b=128)
                    if li == 0:
                        xt_t = gp.tile([128, TPB * 128], BF16, tag="xt_t")
                        nc.sync.dma_start(
                            out=xt_t[:], in_=xT[:, g * TPB * 128:(g + 1) * TPB * 128])
                    for j in range(TPB):
                        t = g * TPB + j
                        if li == 0:
                            lhs = xt_t[:, j * 128:(j + 1) * 128]
                            rhs = wcat_sb[:, 0:h2]
                        else:
                            q = t % 2
                            col = t // 2
                            lhs = hT_sb[q * fi:(q + 1) * fi,
                                        col * 128:(col + 1) * 128]
                            rhs = wcat_sb[q * fi:(q + 1) * fi, 0:h2]
                        ps = gpp.tile([128, h2], FP32, tag="gemm_ps")
                        nc.tensor.matmul(ps[:], lhs, rhs, start=True, stop=True)
                        nc.vector.tensor_tensor(
                            sb3[:, j:j + 1, 0:h2], ps[:].unsqueeze(1),
                            bxr_sb[:, 0:h2].unsqueeze(1), mybir.AluOpType.add)
                    nc.sync.dma_start(out=bat(xl_bounce, g)[:, :, 0:h],
                                      in_=sb3[:, :, 0:h])
                    nc.sync.dma_start(out=bat(xr_tab, g)[:, :, 0:h],
                                      in_=sb3[:, :, h:h2])

                # ---- B: AllGather xl table
                nc.gpsimd.collective_compute(
                    "AllGather", mybir.AluOpType.bypass,
                    replica_groups=[list(range(NCORES))],
                    ins=[xl_bounce[:].opt()], outs=[xl_full[:].opt()],
                )

                # ---- C: zero accumulators
                for r in range(RACC):
                    for k in range(PADSH // 1792):
                        acc3 = accs[r][k * 1792:(k + 1) * 1792, :].rearrange(
                            "(a p) b -> p a b", p=128)
                        nc.sync.dma_start(out=acc3, in_=z3)

                # ---- D: edge windows (2-window compute groups)
                for b in range(NBLK):
                    w0, wcnt = bstart[b]
                    c0 = w0 * (WCAP // 16)
                    si_b = ixp.tile([128, wcnt * WCAP // 16], I16, tag="si_b")
                    nc.sync.dma_start(
                        out=si_b[:], in_=sidx_rep[:, c0:c0 + wcnt * WCAP // 16])
                    di_b = ixp.tile([128, wcnt * WCAP // 16], I16, tag="di_b")
                    nc.sync.dma_start(
                        out=di_b[:], in_=didx_rep[:, c0:c0 + wcnt * WCAP // 16])
                    for g0 in range(0, wcnt, 2):
                        gw = min(2, wcnt - g0)
                        xlg = wp.tile([128, 2 * WT * TW], FP32, tag="xlg")
                        xl4 = xlg[:].rearrange("p (g a b) -> p g a b", g=2, b=TW)
                        xrg = wp.tile([128, 2 * WT * TW], FP32, tag="xrg")
                        xr4 = xrg[:].rearrange("p (g a b) -> p g a b", g=2, b=TW)
                        pay = wp.tile([128, 2 * WT * EL], FP32, tag="pay")
                        p4 = pay[:].rearrange("p (g a b) -> p g a b", g=2, b=EL)
                        e2 = wp.tile([128, 2 * WT], FP32, tag="e2")
                        for k in range(gw):
                            ic0 = (g0 + k) * (WCAP // 16)
                            nc.gpsimd.dma_gather(
                                out_ap=xl4[:, k],
                                in_ap=xl_full[b * BLOCK:b * BLOCK + blk_rows[b], :],
                                idxs_ap=si_b[:, ic0:ic0 + WCAP // 16],
                                num_idxs=WCAP, num_idxs_reg=WCAP,
                                elem_size=TW, single_packet=False,
                            )
                            nc.gpsimd.dma_gather(
                                out_ap=xr4[:, k], in_ap=xr_tab[:],
                                idxs_ap=di_b[:, ic0:ic0 + WCAP // 16],
                                num_idxs=WCAP, num_idxs_reg=WCAP,
                                elem_size=TW, single_packet=False,
                            )
                        gsl = slice(0, gw)
                        e2v = e2[:, 0:gw * WT].rearrange("p (g a) -> p g a", g=gw)
                        nc.vector.tensor_tensor(
                            xr4[:, gsl, :, 0:h], xl4[:, gsl, :, 0:h],
                            xr4[:, gsl, :, 0:h], mybir.AluOpType.add)
                        nc.scalar.activation(
                            p4[:, gsl, :, 0:h], xr4[:, gsl, :, 0:h],
                            mybir.ActivationFunctionType.Copy, scale=NEG_SLOPE)
                        nc.vector.tensor_tensor(
                            xr4[:, gsl, :, 0:h], xr4[:, gsl, :, 0:h],
                            p4[:, gsl, :, 0:h], mybir.AluOpType.max)
                        nc.vector.tensor_tensor(
                            xr4[:, gsl, :, 0:h], xr4[:, gsl, :, 0:h],
                            attr_sb[:].unsqueeze(1).unsqueeze(1)
                                .broadcast_to((128, gw, WT, h)),
                            mybir.AluOpType.mult)
                        nc.vector.tensor_reduce(
                            e2v, xr4[:, gsl, :, 0:h], axis=mybir.AxisListType.X,
                            op=mybir.AluOpType.add)
                        nc.scalar.activation(
                            p4[:, gsl, :, h:EL], e2v.unsqueeze(3),
                            mybir.ActivationFunctionType.Exp)
                        nc.vector.tensor_tensor(
                            p4[:, gsl, :, 0:h], xl4[:, gsl, :, 0:h],
                            p4[:, gsl, :, h:EL].broadcast_to((128, gw, WT, h)),
                            mybir.AluOpType.mult)
                        for k in range(gw):
                            w = w0 + g0 + k
                            ic0 = (g0 + k) * (WCAP // 16)
                            nc.gpsimd.dma_scatter_add(
                                accs[w % RACC][:, 0:EL], p4[:, k],
                                di_b[:, ic0:ic0 + WCAP // 16],
                                WCAP, WCAP, EL, elem_step=ACCW,
                                single_packet=False,
                            )

                # ---- E: merge accs, divide, bias, stats; spill out_pre->acc0
                st2 = ep.tile([128, h2], FP32, tag="st2")
                nc.vector.memset(st2[:], 0.0)
                for g in range(NB):
                    at = ep.tile([128, TPB * ACCW], FP32, tag="at")
                    at3 = at[:].rearrange("p (a b) -> p a b", b=ACCW)
                    nc.sync.dma_start(out=at3, in_=bat(accs[0], g))
                    ar = ep.tile([128, TPB * ACCW], FP32, tag="ar")
                    ar3 = ar[:].rearrange("p (a b) -> p a b", b=ACCW)
                    nc.sync.dma_start(out=ar3, in_=bat(accs[1], g))
                    nc.vector.tensor_tensor(at3[:, :, 0:EL], at3[:, :, 0:EL],
                                            ar3[:, :, 0:EL], mybir.AluOpType.add)
                    rec = ep.tile([128, TPB], FP32, tag="rec")
                    nc.vector.tensor_scalar_add(rec[:].unsqueeze(2),
                                                at3[:, :, h:EL], 1e-30)
                    nc.vector.reciprocal(rec[:], rec[:])
                    op = ep.tile([128, TPB * h], FP32, tag="op")
                    op3 = op[:].rearrange("p (a b) -> p a b", b=h)
                    nc.vector.tensor_tensor(
                        op3, at3[:, :, 0:h],
                        rec[:].unsqueeze(2).broadcast_to((128, TPB, h)),
                        mybir.AluOpType.mult)
                    nc.vector.tensor_tensor(
                        op3, op3,
                        bout_sb[:].unsqueeze(1).broadcast_to((128, TPB, h)),
                        mybir.AluOpType.add)
                    if g == NB - 1:
                        nc.vector.tensor_scalar_mul(
                            op3[:, TPB - 1:TPB, :], op3[:, TPB - 1:TPB, :],
                            padmask_sb[:])
                    red = ep.tile([128, h], FP32, tag="red")
                    nc.vector.tensor_reduce(
                        red[:], op[:].rearrange("p (a b) -> p b a", b=h),
                        axis=mybir.AxisListType.X, op=mybir.AluOpType.add)
                    nc.vector.tensor_tensor(st2[:, 0:h], st2[:, 0:h], red[:],
                                            mybir.AluOpType.add)
                    sq = ep.tile([128, TPB * h], FP32, tag="sq")
                    nc.scalar.activation(sq[:], op[:],
                                         mybir.ActivationFunctionType.Square)
                    red2 = ep.tile([128, h], FP32, tag="red2")
                    nc.vector.tensor_reduce(
                        red2[:], sq[:].rearrange("p (a b) -> p b a", b=h),
                        axis=mybir.AxisListType.X, op=mybir.AluOpType.add)
                    nc.vector.tensor_tensor(st2[:, h:h2], st2[:, h:h2], red2[:],
                                            mybir.AluOpType.add)
                    nc.sync.dma_start(out=bat(accs[0], g)[:, :, 0:h], in_=op3)

                # ---- BN: partition-reduce + AllReduce + a,b rows
                stp = spp.tile([1, h2], FP32, tag="stats_ps")
                nc.tensor.matmul(stp[:], ones_sb[:], st2[:], start=True, stop=True)
                st_sb = ep.tile([1, 128], FP32, tag="st_sb")
                nc.vector.memset(st_sb[:], 0.0)
                nc.scalar.copy(st_sb[0:1, 0:h2], stp[:])
                nc.sync.dma_start(out=bn_in[:], in_=st_sb[:])
                nc.gpsimd.collective_compute(
                    "AllReduce", mybir.AluOpType.add,
                    replica_groups=[list(range(NCORES))],
                    ins=[bn_in[:].opt()], outs=[bn_out[:].opt()],
                )
                stg = ep.tile([1, 128], FP32, tag="stg")
                nc.sync.dma_start(out=stg[:], in_=bn_out[:])
                mu = ep.tile([1, h], FP32, tag="mu")
                nc.vector.tensor_scalar_mul(mu[:], stg[0:1, 0:h], 1.0 / N0)
                var = ep.tile([1, h], FP32, tag="var")
                nc.vector.tensor_scalar_mul(var[:], stg[0:1, h:h2], 1.0 / N0)
                musq = ep.tile([1, h], FP32, tag="musq")
                nc.scalar.activation(musq[:], mu[:], mybir.ActivationFunctionType.Square)
                nc.vector.tensor_tensor(var[:], var[:], musq[:], mybir.AluOpType.subtract)
                nc.vector.tensor_scalar_add(var[:], var[:], EPS)
                nc.scalar.activation(var[:], var[:], mybir.ActivationFunctionType.Sqrt)
                nc.vector.reciprocal(var[:], var[:])
                arow = ep.tile([1, h], FP32, tag="arow")
                nc.vector.tensor_tensor(arow[:], var[:], gam_sb[:], mybir.AluOpType.mult)
                brow = ep.tile([1, h], FP32, tag="brow")
                nc.vector.tensor_tensor(brow[:], mu[:], arow[:], mybir.AluOpType.mult)
                nc.vector.tensor_tensor(brow[:], bet_sb[:], brow[:], mybir.AluOpType.subtract)
                abp = spp.tile([128, h], FP32, tag="ab_ps")
                nc.tensor.matmul(abp[:], ones_row[:], arow[:], start=True, stop=True)
                a_bc = ep.tile([128, h], FP32, tag="a_bc")
                nc.scalar.copy(a_bc[:], abp[:])
                abp2 = spp.tile([128, h], FP32, tag="ab_ps")
                nc.tensor.matmul(abp2[:], ones_row[:], brow[:], start=True, stop=True)
                b_bc = ep.tile([128, h], FP32, tag="b_bc")
                nc.scalar.copy(b_bc[:], abp2[:])

                # ---- F: reload, normalize + relu (batched)
                for g in range(NB):
                    op = ep.tile([128, TPB * h], FP32, tag="op")
                    op3 = op[:].rearrange("p (a b) -> p a b", b=h)
                    nc.sync.dma_start(out=op3, in_=bat(accs[0], g)[:, :, 0:h])
                    nc.vector.tensor_tensor(
                        op3, op3, a_bc[:].unsqueeze(1).broadcast_to((128, TPB, h)),
                        mybir.AluOpType.mult)
                    nc.vector.tensor_tensor(
                        op3, op3, b_bc[:].unsqueeze(1).broadcast_to((128, TPB, h)),
                        mybir.AluOpType.add)
                    if li < 2:
                        nc.scalar.activation(
                            h_bf[:, g * TPB * h:(g + 1) * TPB * h], op[:],
                            mybir.ActivationFunctionType.Relu)
                    else:
                        hp4 = hpool_sb[:, g * TPB * 17:(g + 1) * TPB * 17] \
                            .rearrange("p (a b) -> p a b", b=17)
                        nc.scalar.activation(hp4[:, :, 0:16], op3,
                                             mybir.ActivationFunctionType.Relu)
                        nc.vector.memset(hp4[:, :, 16:17], 1.0)

                # ---- G: paired transposes -> hT_sb for the next layer
                if li < 2:
                    hn = h                      # next layer F_in == h
                    for k in range(NT // 2):
                        t0 = k * 2
                        ncols = 2 * hn
                        tp = gpp.tile([128, 128], BF16, tag="tr_ps")
                        nc.tensor.transpose(
                            tp[0:ncols, :],
                            h_bf[:, t0 * hn:t0 * hn + ncols],
                            ident_sb[:])
                        nc.scalar.copy(hT_sb[0:ncols, k * 128:(k + 1) * 128],
                                       tp[0:ncols, :])

            # ---- pooling: indicator matmuls accumulate [G, 17]
            plp = spp.tile([G, 17], FP32, tag="pool_ps")
            for t in range(NT):
                ind = gp.tile([128, G], FP32, tag="ind")
                nc.vector.tensor_tensor(
                    ind[:], batch_sb[:, t:t + 1].broadcast_to((128, G)),
                    iota_sb[:], mybir.AluOpType.is_equal)
                nc.tensor.matmul(plp[:], ind[:],
                                 hpool_sb[:, t * 17:(t + 1) * 17],
                                 start=(t == 0), stop=(t == NT - 1))
            pl_sb = gp.tile([G, 17], FP32, tag="pl_sb")
            nc.scalar.copy(pl_sb[:], plp[:])
            nc.sync.dma_start(out=pool_out[:], in_=pl_sb[:])

    nc.compile()
    return nc


# ---------------------------------------------------------------- kernel()

def kernel(**inputs):
    in_maps, win_bucket = _make_in_maps(inputs)
    nc = _build_nc(win_bucket)

    global LAST_EXEC_NS
    t0 = time.time()
    res = run_bass_kernel_spmd(nc, in_maps, core_ids=list(range(NCORES)))
    LAST_EXEC_NS = (time.time() - t0) * 1e9

    pool = np.zeros((G, 17), np.float64)
    for c in range(NCORES):
        pool += np.asarray(res.results[c]["pool"], np.float64)
    sums, cnt = pool[:, :16], pool[:, 16]
    pooled = sums / np.maximum(cnt, 1.0)[:, None]
    linW = np.asarray(inputs["linW"], np.float32)
    linb = np.asarray(inputs["linb"], np.float32)
    return (pooled.astype(np.float32) @ linW + linb).astype(np.float32)


# revision 19
# speedup vs baseline: 3.7949x; 3.7949x over previous
"""GATv2 3-layer GNN (nn_GCN_10917806866525) on 8 TRN2 NeuronCores.

Sharding: nodes split 12500/core (edge-cut by dst). Per layer, per core:
  A. node-transform GEMM for the local shard (merged [Wl|Wr], act-stationary
     bf16 matmul; stationary tiles packed into partition halves/quarters for
     layers 2/3) -> xl shard + local xr table
  B. AllGather of the xl shard -> full xl table [100352, 64] f32
  C. edge phase over 4096-token windows (host-packed so every window has
     unique dst -> dma_scatter_add duplicate-safety), processed in 2-window
     groups: 2 dma_gathers/window (xl[src], xr[dst]), LeakyReLU via
     scale-copy+max (ACT Lrelu mis-evaluates on HW), att-dot, exp written
     straight into the payload's w column, scatter of [w*xl | w] into 2
     rotating DRAM accumulators
  D. merge accs in 7-tile batches, divide by the w column (softmax
     denominator; max-subtraction skipped, exponents small in fp32),
     + (bl+bias), BN stats via ones-matmul + AllReduce [1,2H], spill pre-BN
     out to acc0
  E. reload, normalize+ReLU (batched), paired PE transposes -> next layer's
     stationary operand
Pooling: per-tile indicator matmuls accumulated in PSUM -> [64,17] partials
per core; host sums cores, divides counts, applies the final linear layer.

Everything is sized to minimize instruction count: the axon execute path
costs ~35-65us per instruction, dwarfing actual engine time."""

import time

import numpy as np
import ml_dtypes

import concourse.bacc as bacc
import concourse.bass as bass
import concourse.mybir as mybir
from concourse import tile
from concourse.bass_utils import run_bass_kernel_spmd

FP32 = mybir.dt.float32
BF16 = mybir.dt.bfloat16
I16 = mybir.dt.int16

NCORES = 8
N = 100000
F = 128
E = 1600000
G = 64
EPS = 1e-5
NEG_SLOPE = 0.2

SHARD = N // NCORES          # 12500
PADSH = 12544                # 98 * 128
NT = PADSH // 128            # 98 node tiles per shard
NB = 14                      # 7-tile epilogue batches
TPB = 7                      # tiles per batch
TABN = NCORES * PADSH        # 100352 rows in the all-gathered xl table
BLOCK = 32768                # int16 gather block
NBLK = (TABN + BLOCK - 1) // BLOCK   # 4
WCAP = 4096                  # tokens per window (= one gather/scatter op)
WT = WCAP // 128             # 32 token columns per window
RACC = 2                     # rotating scatter accumulators
N0 = float(N)

LAYERS = [(128, 64), (64, 32), (32, 16)]  # (F_in, H)
TW = 64                      # gather table width (f32, 256B rows)
ACCW = 128                   # accumulator row stride (512B)

LAST_EXEC_NS = None


# ---------------------------------------------------------------- host prep

def _wrap16(idx):
    """[n] int -> [16, n/16] int16 wrapped (idx j at [j%16, j//16])."""
    return np.ascontiguousarray(np.asarray(idx, np.int16).reshape(-1, 16).T)


def _build_windows(edge_index):
    """Partition edges by dst core; per core, bucket by src table block and
    pack into 4096-token windows with per-window-unique dst_local. Dummy
    tokens (src row 0 of the bucket, dst pad row 12500) fill windows so all
    cores share one SPMD-identical window layout."""
    src = np.concatenate([edge_index[0], np.arange(N, dtype=np.int64)])
    dst = np.concatenate([edge_index[1], np.arange(N, dtype=np.int64)])
    core = dst // SHARD
    dst_local = (dst - core * SHARD).astype(np.int64)
    src_tab = (src // SHARD) * PADSH + (src % SHARD)
    bucket = src_tab // BLOCK
    src_blk = (src_tab - bucket * BLOCK).astype(np.int64)

    per = [[None] * NBLK for _ in range(NCORES)]
    for c in range(NCORES):
        mc = core == c
        for b in range(NBLK):
            m = mc & (bucket == b)
            per[c][b] = (src_blk[m], dst_local[m])

    wb = []
    for b in range(NBLK):
        need = 0
        for c in range(NCORES):
            s, d = per[c][b]
            cnt = np.bincount(d, minlength=SHARD)
            need = max(need, int(np.ceil(len(d) / (WCAP - 96))) + 1, int(cnt.max()))
        wb.append(need)

    srcidx = [[] for _ in range(NCORES)]
    dstidx = [[] for _ in range(NCORES)]
    win_bucket = []
    for b in range(NBLK):
        W = wb[b]
        win_bucket += [b] * W
        for c in range(NCORES):
            s, d = per[c][b]
            order = np.argsort(d, kind="stable")
            s, d = s[order], d[order]
            occ = np.arange(len(d)) - np.searchsorted(d, d, side="left")
            w_of = (d + occ) % W
            counts = np.bincount(w_of, minlength=W)
            if counts.max() > WCAP:
                has = np.zeros((W, SHARD + 1), bool)
                has[w_of, d] = True
                for w in np.where(counts > WCAP)[0]:
                    idxs = np.where(w_of == w)[0]
                    for i in idxs[WCAP:]:
                        cand = np.where((counts < WCAP) & ~has[:, d[i]])[0]
                        if len(cand) == 0:
                            raise RuntimeError("window spill placement failed")
                        has[w, d[i]] = False
                        has[cand[0], d[i]] = True
                        counts[w] -= 1
                        counts[cand[0]] += 1
                        w_of[i] = cand[0]
            o2 = np.lexsort((d, w_of))
            s2, d2, w2 = s[o2], d[o2], w_of[o2]
            starts = np.concatenate([[0], np.cumsum(counts)[:-1]])
            pos = np.arange(len(d2)) - np.repeat(starts, counts)
            out_s = np.zeros((W, WCAP), np.int64)
            out_d = np.full((W, WCAP), SHARD, np.int64)
            out_s[w2, pos] = s2
            out_d[w2, pos] = d2
            srcidx[c].append(out_s.reshape(-1))
            dstidx[c].append(out_d.reshape(-1))

    src_w = [_wrap16(np.concatenate(srcidx[c])) for c in range(NCORES)]
    dst_w = [_wrap16(np.concatenate(dstidx[c])) for c in range(NCORES)]
    return src_w, dst_w, win_bucket


def _make_in_maps(inputs):
    x = np.asarray(inputs["x"], np.float32)
    ei = np.asarray(inputs["edge_index"], np.int64)
    batch = np.asarray(inputs["batch"], np.int64)
    src_w, dst_w, win_bucket = _build_windows(ei)

    xT = np.zeros((NCORES, 128, PADSH), ml_dtypes.bfloat16)
    xt_full = x.T.astype(ml_dtypes.bfloat16)
    for c in range(NCORES):
        xT[c, :, :SHARD] = xt_full[:, c * SHARD:(c + 1) * SHARD]

    ident = np.eye(128, dtype=ml_dtypes.bfloat16)
    iota = np.tile(np.arange(G, dtype=np.float32)[None, :], (128, 1))
    batchf = np.full((NCORES, 128, NT), -1.0, np.float32)
    for c in range(NCORES):
        loc = batch[c * SHARD:(c + 1) * SHARD].astype(np.float32)
        pad = np.full(PADSH - SHARD, -1.0, np.float32)
        batchf[c] = np.concatenate([loc, pad]).reshape(NT, 128).T

    const_maps = {"ident": ident, "iota": iota}
    for li, (fi, h) in enumerate(LAYERS):
        Wl = np.asarray(inputs[f"Wl{li+1}"], np.float32)
        bl = np.asarray(inputs[f"bl{li+1}"], np.float32)
        Wr = np.asarray(inputs[f"Wr{li+1}"], np.float32)
        br = np.asarray(inputs[f"br{li+1}"], np.float32)
        att = np.asarray(inputs[f"att{li+1}"], np.float32)
        bias = np.asarray(inputs[f"bias{li+1}"], np.float32)
        gam = np.asarray(inputs[f"gamma{li+1}"], np.float32)
        bet = np.asarray(inputs[f"beta{li+1}"], np.float32)
        wcat = np.concatenate([Wl, Wr], 1).astype(ml_dtypes.bfloat16)  # [fi, 2h]
        rep = 128 // fi
        const_maps[f"wcat{li}"] = np.ascontiguousarray(np.tile(wcat, (rep, 1)))
        const_maps[f"attr{li}"] = np.tile(att[None, :], (128, 1)).astype(np.float32)
        const_maps[f"bout{li}"] = np.tile((bl + bias)[None, :], (128, 1)).astype(np.float32)
        bxr = np.concatenate([np.zeros(h, np.float32), bl + br])
        const_maps[f"bxr{li}"] = np.tile(bxr[None, :], (128, 1)).astype(np.float32)
        const_maps[f"gam{li}"] = gam[None, :].astype(np.float32)
        const_maps[f"bet{li}"] = bet[None, :].astype(np.float32)

    padmask = (np.arange(128) + (NT - 1) * 128 < SHARD).astype(np.float32)[:, None]
    in_maps = [
        {"xT": xT[c], "sidx": src_w[c], "didx": dst_w[c], "batchf": batchf[c],
         "padmask": padmask, **const_maps}
        for c in range(NCORES)
    ]
    return in_maps, win_bucket


# ---------------------------------------------------------------- device nc

def _build_nc(win_bucket):
    TOTW = len(win_bucket)
    TOT16 = TOTW * WCAP // 16
    bstart = {}
    for w, b in enumerate(win_bucket):
        if b not in bstart:
            bstart[b] = [w, 0]
        bstart[b][1] += 1

    nc = bacc.Bacc("TRN2", target_bir_lowering=False, debug=False,
                   enable_asserts=False, num_devices=NCORES)

    xT = nc.declare_dram_parameter("xT", [128, PADSH], BF16, isOutput=False)
    sidx_in = nc.declare_dram_parameter("sidx", [16, TOT16], I16, isOutput=False)
    didx_in = nc.declare_dram_parameter("didx", [16, TOT16], I16, isOutput=False)
    ident_in = nc.declare_dram_parameter("ident", [128, 128], BF16, isOutput=False)
    iota_in = nc.declare_dram_parameter("iota", [128, G], FP32, isOutput=False)
    batch_in = nc.declare_dram_parameter("batchf", [128, NT], FP32, isOutput=False)
    padmask_in = nc.declare_dram_parameter("padmask", [128, 1], FP32, isOutput=False)
    wcat_in, attr_in, bout_in, bxr_in, gam_in, bet_in = [], [], [], [], [], []
    for li, (fi, h) in enumerate(LAYERS):
        wcat_in.append(nc.declare_dram_parameter(f"wcat{li}", [128, 2 * h], BF16, isOutput=False))
        attr_in.append(nc.declare_dram_parameter(f"attr{li}", [128, h], FP32, isOutput=False))
        bout_in.append(nc.declare_dram_parameter(f"bout{li}", [128, h], FP32, isOutput=False))
        bxr_in.append(nc.declare_dram_parameter(f"bxr{li}", [128, 2 * h], FP32, isOutput=False))
        gam_in.append(nc.declare_dram_parameter(f"gam{li}", [1, h], FP32, isOutput=False))
        bet_in.append(nc.declare_dram_parameter(f"bet{li}", [1, h], FP32, isOutput=False))
    pool_out = nc.declare_dram_parameter("pool", [G, 17], FP32, isOutput=True)

    xl_bounce = nc.dram_tensor("xl_bounce", [PADSH, TW], FP32)
    xl_full = nc.dram_tensor("xl_full", [TABN, TW], FP32, addr_space="Shared")
    xr_tab = nc.dram_tensor("xr_tab", [PADSH, TW], FP32)
    accs = [nc.dram_tensor(f"acc{r}", [PADSH, ACCW], FP32) for r in range(RACC)]
    sidx_rep = nc.dram_tensor("sidx_rep", [128, TOT16], I16)
    didx_rep = nc.dram_tensor("didx_rep", [128, TOT16], I16)
    bn_in = nc.dram_tensor("bn_in", [1, 128], FP32)
    bn_out = nc.dram_tensor("bn_out", [1, 128], FP32, addr_space="Shared")

    blk_rows = [min(BLOCK, TABN - b * BLOCK) for b in range(NBLK)]

    def bat(dram, g):
        """rows g*896.. as [128, 7, w] (node (7g+j)*128+p -> [p, j, :])."""
        return dram[g * 896:(g + 1) * 896, :].rearrange("(a p) c -> p a c", p=128)

    with tile.TileContext(nc) as tc:
        with (
            tc.tile_pool(name="persist", bufs=1) as pp,
            tc.tile_pool(name="gemm", bufs=3) as gp,
            tc.tile_pool(name="gpsum", bufs=2, space="PSUM") as gpp,
            tc.tile_pool(name="spsum", bufs=1, space="PSUM") as spp,
            tc.tile_pool(name="win", bufs=2) as wp,
            tc.tile_pool(name="ixp", bufs=1) as ixp,
            tc.tile_pool(name="ep", bufs=2) as ep,
            tc.tile_pool(name="misc", bufs=2) as mp,
        ):
            # ---- persistent loads / setup
            ident_sb = pp.tile([128, 128], BF16)
            nc.sync.dma_start(out=ident_sb[:], in_=ident_in[:])
            iota_sb = pp.tile([128, G], FP32)
            nc.sync.dma_start(out=iota_sb[:], in_=iota_in[:])
            batch_sb = pp.tile([128, NT], FP32)
            nc.sync.dma_start(out=batch_sb[:], in_=batch_in[:])
            padmask_sb = pp.tile([128, 1], FP32)
            nc.sync.dma_start(out=padmask_sb[:], in_=padmask_in[:])
            ones_sb = pp.tile([128, 1], FP32)
            nc.vector.memset(ones_sb[:], 1.0)
            ones_row = pp.tile([1, 128], FP32)
            nc.vector.memset(ones_row[:], 1.0)
            zero_sb = pp.tile([128, 1792], FP32)
            nc.vector.memset(zero_sb[:], 0.0)
            z3 = zero_sb[:].rearrange("p (a b) -> p a b", b=128)
            for k in range(8):
                nc.sync.dma_start(out=sidx_rep[16 * k:16 * (k + 1), :], in_=sidx_in[:])
                nc.sync.dma_start(out=didx_rep[16 * k:16 * (k + 1), :], in_=didx_in[:])

            hT_sb = pp.tile([128, (NT // 2 + 1) * 128], BF16)
            h_bf = pp.tile([128, NT * TW], BF16, tag="hshare")
            hpool_sb = pp.tile([128, NT * 17], BF16)

            for li, (fi, h) in enumerate(LAYERS):
                h2 = 2 * h
                EL = h + 1
                rep = 128 // fi
                # ---- layer consts
                wcat_sb = mp.tile([128, h2], BF16, tag="wcat")
                nc.sync.dma_start(out=wcat_sb[:], in_=wcat_in[li][:])
                attr_sb = mp.tile([128, h], FP32, tag="attr")
                nc.sync.dma_start(out=attr_sb[:], in_=attr_in[li][:])
                bout_sb = mp.tile([128, h], FP32, tag="bout")
                nc.sync.dma_start(out=bout_sb[:], in_=bout_in[li][:])
                bxr_sb = mp.tile([128, h2], FP32, tag="bxr")
                nc.sync.dma_start(out=bxr_sb[:], in_=bxr_in[li][:])
                gam_sb = mp.tile([1, h], FP32, tag="gam")
                nc.sync.dma_start(out=gam_sb[:], in_=gam_in[li][:])
                bet_sb = mp.tile([1, h], FP32, tag="bet")
                nc.sync.dma_start(out=bet_sb[:], in_=bet_in[li][:])

                # ---- A: GEMM -> xl_bounce + xr_tab (7-tile output batches)
                for g in range(NB):
                    sb = gp.tile([128, TPB * 128], FP32, tag="gemm_sb")
                    sb3 = sb[:].rearrange("p (a b) -> p a b", b=128)
                    if li == 0:
                        xt_t = gp.tile([128, TPB * 128], BF16, tag="xt_t")
                        nc.sync.dma_start(
                            out=xt_t[:], in_=xT[:, g * TPB * 128:(g + 1) * TPB * 128])
                    for j in range(TPB):
                        t = g * TPB + j
                        if li == 0:
                            lhs = xt_t[:, j * 128:(j + 1) * 128]
                            rhs = wcat_sb[:, 0:h2]
                        else:
                            q = t % 2
                            col = t // 2
                            lhs = hT_sb[q * fi:(q + 1) * fi,
                                        col * 128:(col + 1) * 128]
                            rhs = wcat_sb[q * fi:(q + 1) * fi, 0:h2]
                        ps = gpp.tile([128, h2], FP32, tag="gemm_ps")
                        nc.tensor.matmul(ps[:], lhs, rhs, start=True, stop=True)
                        nc.vector.tensor_tensor(
                            sb3[:, j:j + 1, 0:h2], ps[:].unsqueeze(1),
                            bxr_sb[:, 0:h2].unsqueeze(1), mybir.AluOpType.add)
                    nc.sync.dma_start(out=bat(xl_bounce, g)[:, :, 0:h],
                                      in_=sb3[:, :, 0:h])
                    nc.sync.dma_start(out=bat(xr_tab, g)[:, :, 0:h],
                                      in_=sb3[:, :, h:h2])

                # ---- B: AllGather xl table
                nc.gpsimd.collective_compute(
                    "AllGather", mybir.AluOpType.bypass,
                    replica_groups=[list(range(NCORES))],
                    ins=[xl_bounce[:].opt()], outs=[xl_full[:].opt()],
                )

                # ---- C: zero accumulators
                for r in range(RACC):
                    for k in range(PADSH // 1792):
                        acc3 = accs[r][k * 1792:(k + 1) * 1792, :].rearrange(
                            "(a p) b -> p a b", p=128)
                        nc.sync.dma_start(out=acc3, in_=z3)

                # ---- D: edge windows (2-window compute groups)
                for b in range(NBLK):
                    w0, wcnt = bstart[b]
                    c0 = w0 * (WCAP // 16)
                    si_b = ixp.tile([128, wcnt * WCAP // 16], I16, tag="si_b")
                    nc.sync.dma_start(
                        out=si_b[:], in_=sidx_rep[:, c0:c0 + wcnt * WCAP // 16])
                    di_b = ixp.tile([128, wcnt * WCAP // 16], I16, tag="di_b")
                    nc.sync.dma_start(
                        out=di_b[:], in_=didx_rep[:, c0:c0 + wcnt * WCAP // 16])
                    for g0 in range(0, wcnt, 2):
                        gw = min(2, wcnt - g0)
                        xlg = wp.tile([128, 2 * WT * TW], FP32, tag="xlg")
                        xl4 = xlg[:].rearrange("p (g a b) -> p g a b", g=2, b=TW)
                        xrg = wp.tile([128, 2 * WT * TW], FP32, tag="xrg")
                        xr4 = xrg[:].rearrange("p (g a b) -> p g a b", g=2, b=TW)
                        pay = wp.tile([128, 2 * WT * EL], FP32, tag="pay")
                        p4 = pay[:].rearrange("p (g a b) -> p g a b", g=2, b=EL)
                        e2 = wp.tile([128, 2 * WT], FP32, tag="e2")
                        for k in range(gw):
                            ic0 = (g0 + k) * (WCAP // 16)
                            nc.gpsimd.dma_gather(
                                out_ap=xl4[:, k],
                                in_ap=xl_full[b * BLOCK:b * BLOCK + blk_rows[b], :],
                                idxs_ap=si_b[:, ic0:ic0 + WCAP // 16],
                                num_idxs=WCAP, num_idxs_reg=WCAP,
                                elem_size=TW, single_packet=False,
                            )
                            nc.gpsimd.dma_gather(
                                out_ap=xr4[:, k], in_ap=xr_tab[:],
                                idxs_ap=di_b[:, ic0:ic0 + WCAP // 16],
                                num_idxs=WCAP, num_idxs_reg=WCAP,
                                elem_size=TW, single_packet=False,
                            )
                        gsl = slice(0, gw)
                        e2v = e2[:, 0:gw * WT].rearrange("p (g a) -> p g a", g=gw)
                        nc.vector.tensor_tensor(
                            xr4[:, gsl, :, 0:h], xl4[:, gsl, :, 0:h],
                            xr4[:, gsl, :, 0:h], mybir.AluOpType.add)
                        nc.scalar.activation(
                            p4[:, gsl, :, 0:h], xr4[:, gsl, :, 0:h],
                            mybir.ActivationFunctionType.Copy, scale=NEG_SLOPE)
                        nc.vector.tensor_tensor(
                            xr4[:, gsl, :, 0:h], xr4[:, gsl, :, 0:h],
                            p4[:, gsl, :, 0:h], mybir.AluOpType.max)
                        nc.vector.tensor_tensor(
                            xr4[:, gsl, :, 0:h], xr4[:, gsl, :, 0:h],
                            attr_sb[:].unsqueeze(1).unsqueeze(1)
                                .broadcast_to((128, gw, WT, h)),
                            mybir.AluOpType.mult)
                        nc.vector.tensor_reduce(
                            e2v, xr4[:, gsl, :, 0:h], axis=mybir.AxisListType.X,
                            op=mybir.AluOpType.add)
                        nc.scalar.activation(
                            p4[:, gsl, :, h:EL], e2v.unsqueeze(3),
                            mybir.ActivationFunctionType.Exp)
                        nc.vector.tensor_tensor(
                            p4[:, gsl, :, 0:h], xl4[:, gsl, :, 0:h],
                            p4[:, gsl, :, h:EL].broadcast_to((128, gw, WT, h)),
                            mybir.AluOpType.mult)
                        for k in range(gw):
                            w = w0 + g0 + k
                            ic0 = (g0 + k) * (WCAP // 16)
                            nc.gpsimd.dma_scatter_add(
                                accs[w % RACC][:, 0:EL], p4[:, k],
                                di_b[:, ic0:ic0 + WCAP // 16],
                                WCAP, WCAP, EL, elem_step=ACCW,
                                single_packet=False,
                            )

                # ---- E: merge accs, divide, bias, stats; spill out_pre->acc0
                st2 = ep.tile([128, h2], FP32, tag="st2")
                nc.vector.memset(st2[:], 0.0)
                for g in range(NB):
                    at = ep.tile([128, TPB * ACCW], FP32, tag="at")
                    at3 = at[:].rearrange("p (a b) -> p a b", b=ACCW)
                    nc.sync.dma_start(out=at3, in_=bat(accs[0], g))
                    ar = ep.tile([128, TPB * ACCW], FP32, tag="ar")
                    ar3 = ar[:].rearrange("p (a b) -> p a b", b=ACCW)
                    nc.sync.dma_start(out=ar3, in_=bat(accs[1], g))
                    nc.vector.tensor_tensor(at3[:, :, 0:EL], at3[:, :, 0:EL],
                                            ar3[:, :, 0:EL], mybir.AluOpType.add)
                    rec = ep.tile([128, TPB], FP32, tag="rec")
                    nc.vector.tensor_scalar_add(rec[:].unsqueeze(2),
                                                at3[:, :, h:EL], 1e-30)
                    nc.vector.reciprocal(rec[:], rec[:])
                    op = ep.tile([128, TPB * h], FP32, tag="op")
                    op3 = op[:].rearrange("p (a b) -> p a b", b=h)
                    nc.vector.tensor_tensor(
                        op3, at3[:, :, 0:h],
                        rec[:].unsqueeze(2).broadcast_to((128, TPB, h)),
                        mybir.AluOpType.mult)
                    nc.vector.tensor_tensor(
                        op3, op3,
                        bout_sb[:].unsqueeze(1).broadcast_to((128, TPB, h)),
                        mybir.AluOpType.add)
                    if g == NB - 1:
                        nc.vector.tensor_scalar_mul(
                            op3[:, TPB - 1:TPB, :], op3[:, TPB - 1:TPB, :],
                            padmask_sb[:])
                    red = ep.tile([128, h], FP32, tag="red")
                    nc.vector.tensor_reduce(
                        red[:], op[:].rearrange("p (a b) -> p b a", b=h),
                        axis=mybir.AxisListType.X, op=mybir.AluOpType.add)
                    nc.vector.tensor_tensor(st2[:, 0:h], st2[:, 0:h], red[:],
                                            mybir.AluOpType.add)
                    sq = ep.tile([128, TPB * h], FP32, tag="sq")
                    nc.scalar.activation(sq[:], op[:],
                                         mybir.ActivationFunctionType.Square)
                    red2 = ep.tile([128, h], FP32, tag="red2")
                    nc.vector.tensor_reduce(
                        red2[:], sq[:].rearrange("p (a b) -> p b a", b=h),
                        axis=mybir.AxisListType.X, op=mybir.AluOpType.add)
                    nc.vector.tensor_tensor(st2[:, h:h2], st2[:, h:h2], red2[:],
                                            mybir.AluOpType.add)
                    nc.sync.dma_start(out=bat(accs[0], g)[:, :, 0:h], in_=op3)

                # ---- BN: partition-reduce + AllReduce + a,b rows
                stp = spp.tile([1, h2], FP32, tag="stats_ps")
                nc.tensor.matmul(stp[:], ones_sb[:], st2[:], start=True, stop=True)
                st_sb = ep.tile([1, 128], FP32, tag="st_sb")
                nc.vector.memset(st_sb[:], 0.0)
                nc.scalar.copy(st_sb[0:1, 0:h2], stp[:])
                nc.sync.dma_start(out=bn_in[:], in_=st_sb[:])
                nc.gpsimd.collective_compute(
                    "AllReduce", mybir.AluOpType.add,
                    replica_groups=[list(range(NCORES))],
                    ins=[bn_in[:].opt()], outs=[bn_out[:].opt()],
                )
                stg = ep.tile([1, 128], FP32, tag="stg")
                nc.sync.dma_start(out=stg[:], in_=bn_out[:])
                mu = ep.tile([1, h], FP32, tag="mu")
                nc.vector.tensor_scalar_mul(mu[:], stg[0:1, 0:h], 1.0 / N0)
                var = ep.tile([1, h], FP32, tag="var")
                nc.vector.tensor_scalar_mul(var[:], stg[0:1, h:h2], 1.0 / N0)
                musq = ep.tile([1, h], FP32, tag="musq")
                nc.scalar.activation(musq[:], mu[:], mybir.ActivationFunctionType.Square)
                nc.vector.tensor_tensor(var[:], var[:], musq[:], mybir.AluOpType.subtract)
                nc.vector.tensor_scalar_add(var[:], var[:], EPS)
                nc.scalar.activation(var[:], var[:], mybir.ActivationFunctionType.Sqrt)
                nc.vector.reciprocal(var[:], var[:])
                arow = ep.tile([1, h], FP32, tag="arow")
                nc.vector.tensor_tensor(arow[:], var[:], gam_sb[:], mybir.AluOpType.mult)
                brow = ep.tile([1, h], FP32, tag="brow")
                nc.vector.tensor_tensor(brow[:], mu[:], arow[:], mybir.AluOpType.mult)
                nc.vector.tensor_tensor(brow[:], bet_sb[:], brow[:], mybir.AluOpType.subtract)
                abp = spp.tile([128, h], FP32, tag="ab_ps")
                nc.tensor.matmul(abp[:], ones_row[:], arow[:], start=True, stop=True)
                a_bc = ep.tile([128, h], FP32, tag="a_bc")
                nc.scalar.copy(a_bc[:], abp[:])
                abp2 = spp.tile([128, h], FP32, tag="ab_ps")
                nc.tensor.matmul(abp2[:], ones_row[:], brow[:], start=True, stop=True)
                b_bc = ep.tile([128, h], FP32, tag="b_bc")
                nc.scalar.copy(b_bc[:], abp2[:])

                # ---- F: reload, normalize + relu (batched)
                for g in range(NB):
                    op = ep.tile([128, TPB * h], FP32, tag="op")
                    op3 = op[:].rearrange("p (a b) -> p a b", b=h)
                    nc.sync.dma_start(out=op3, in_=bat(accs[0], g)[:, :, 0:h])
                    nc.vector.tensor_tensor(
                        op3, op3, a_bc[:].unsqueeze(1).broadcast_to((128, TPB, h)),
                        mybir.AluOpType.mult)
                    nc.vector.tensor_tensor(
                        op3, op3, b_bc[:].unsqueeze(1).broadcast_to((128, TPB, h)),
                        mybir.AluOpType.add)
                    if li < 2:
                        nc.scalar.activation(
                            h_bf[:, g * TPB * h:(g + 1) * TPB * h], op[:],
                            mybir.ActivationFunctionType.Relu)
                    else:
                        hp4 = hpool_sb[:, g * TPB * 17:(g + 1) * TPB * 17] \
                            .rearrange("p (a b) -> p a b", b=17)
                        nc.scalar.activation(hp4[:, :, 0:16], op3,
                                             mybir.ActivationFunctionType.Relu)
                        nc.vector.memset(hp4[:, :, 16:17], 1.0)

                # ---- G: paired transposes -> hT_sb for the next layer
                if li < 2:
                    hn = h                      # next layer F_in == h
                    for k in range(NT // 2):
                        t0 = k * 2
                        ncols = 2 * hn
                        tp = gpp.tile([128, 128], BF16, tag="tr_ps")
                        nc.tensor.transpose(
                            tp[0:ncols, :],
                            h_bf[:, t0 * hn:t0 * hn + ncols],
                            ident_sb[:])
                        nc.scalar.copy(hT_sb[0:ncols, k * 128:(k + 1) * 128],
                                       tp[0:ncols, :])

            # ---- pooling: indicator matmuls accumulate [G, 17]
            ind_all = pp.tile([128, NT * G], BF16, tag="hshare")
            nc.vector.tensor_tensor(
                ind_all[:].rearrange("p (a b) -> p a b", b=G),
                batch_sb[:].unsqueeze(2).broadcast_to((128, NT, G)),
                iota_sb[:].unsqueeze(1).broadcast_to((128, NT, G)),
                mybir.AluOpType.is_equal)
            plp = spp.tile([G, 17], FP32, tag="pool_ps")
            for t in range(NT):
                nc.tensor.matmul(plp[:], ind_all[:, t * G:(t + 1) * G],
                                 hpool_sb[:, t * 17:(t + 1) * 17],
                                 start=(t == 0), stop=(t == NT - 1))
            pl_sb = gp.tile([G, 17], FP32, tag="pl_sb")
            nc.scalar.copy(pl_sb[:], plp[:])
            nc.sync.dma_start(out=pool_out[:], in_=pl_sb[:])

    nc.compile()
    return nc


# ---------------------------------------------------------------- kernel()

def kernel(**inputs):
    in_maps, win_bucket = _make_in_maps(inputs)
    nc = _build_nc(win_bucket)

    global LAST_EXEC_NS
    t0 = time.time()
    res = run_bass_kernel_spmd(nc, in_maps, core_ids=list(range(NCORES)))
    LAST_EXEC_NS = (time.time() - t0) * 1e9

    pool = np.zeros((G, 17), np.float64)
    for c in range(NCORES):
        pool += np.asarray(res.results[c]["pool"], np.float64)
    sums, cnt = pool[:, :16], pool[:, 16]
    pooled = sums / np.maximum(cnt, 1.0)[:, None]
    linW = np.asarray(inputs["linW"], np.float32)
    linb = np.asarray(inputs["linb"], np.float32)
    return (pooled.astype(np.float32) @ linW + linb).astype(np.float32)
